# revision 1
# baseline (speedup 1.0000x reference)
"""
CrossAttention kernel for 8x Trainium2 NeuronCores (Bass/Tile).

Problem: nn_CrossAttention (B=4, Sq=Skv=1024, DM=1024, H=16, DK=64), fp32 I/O.

Sharding (Megatron-style hybrid, 8 cores = 2 batch-pairs x 4 head-quarters):
  core c -> batches {2*(c//4), 2*(c//4)+1}, heads {4*(c%4) .. 4*(c%4)+3}.
Each core computes Q/K/V projections for its 4 heads (256 of the 1024
Wq/Wk/Wv output dims), attention for those heads, and a partial O
projection (its 256 rows of Wo).  The host sums the 4 head-quarter
partials per batch and adds bo (standard Megatron bias-after-allreduce).

Device-side dataflow (per core), all matmuls bf16 with fp32 PSUM accumulate:
  - Host pre-transposes y1/y2 (feature-major yT) and bias (biasT [Skv,Sq]),
    so no on-device transposes are needed anywhere.
  - qT/kT produced feature-major [dk, Sq]; bq (pre-scaled by 1/sqrt(dk)) and
    bk fused into the PSUM->SBUF evacuation via ACT activation bias.
  - v produced token-major [Skv, dk] with bv added via a rank-1 (ones-row)
    matmul; a ones-column is appended to v so that the PV matmul emits the
    softmax denominator as PSUM row 64 for free (M=65 trick).
  - logits computed transposed l^T [Skv, Sq] = k^T q; bias added on the
    tensor engine as a bf16 identity-matmul accumulated into the same PSUM
    group (identity shipped from host; bias cast to bf16 by gpsimd/DVE);
    exp on ACT straight to bf16 p^T in SBUF.  No max-subtraction:
    |logits| <= ~15 is exact-exp-safe in fp32.
  - attn^T = v^T p^T (unnormalized) + denominator row; normalization applied
    at PSUM evacuation: r = 1/s broadcast across partitions via a DRAM
    round-trip DMA (0-step partition AP), DVE multiply during evacuation.
  - O projection token-major from assembled attn^T head-pair tiles.
"""

import os
import sys

sys.path.insert(0, "/opt/trn_rl_repo")

from contextlib import ExitStack

import numpy as np
import ml_dtypes

import concourse.bass as bass
import concourse.mybir as mybir
import concourse.tile as tile
from concourse import bacc
from concourse.bass_utils import run_bass_kernel_spmd

BF16 = mybir.dt.bfloat16
F32 = mybir.dt.float32
AF = mybir.ActivationFunctionType
ALU = mybir.AluOpType

B, Sq, Skv, DM = 4, 1024, 1024, 1024
H, DK = 16, 64
HC = 4            # heads per core
DH = HC * DK      # head dims per core (256)
BC = 2            # batches per core
SCALE = DK ** -0.5
N_CORES = 8

_PROGRAM = None   # cached (nc, out_name)
TRACE = bool(int(os.environ.get("BASS_KERNEL_TRACE", "0")))
SKIP_BIAS = bool(int(os.environ.get("K_SKIP_BIAS", "0")))
SKIP_NORM = bool(int(os.environ.get("K_SKIP_NORM", "0")))
SKIP_PROJ = bool(int(os.environ.get("K_SKIP_PROJ", "0")))
SKIP_ATTN = bool(int(os.environ.get("K_SKIP_ATTN", "0")))
PE_BIAS = int(os.environ.get("K_PE_BIAS", "0"))  # every Nth bias tile on PE via I-matmul
LPS_BUFS = int(os.environ.get("K_LPS_BUFS", "2"))
APS_BUFS = int(os.environ.get("K_APS_BUFS", "4"))
OPS_BUFS = int(os.environ.get("K_OPS_BUFS", "2"))
LAST_RESULTS = None


def build_program():
    """Build the per-core SPMD Bass program (identical on all 8 cores)."""
    nc = bacc.Bacc(
        "TRN2",
        target_bir_lowering=False,
        debug=False,
        num_devices=N_CORES,
    )

    # ---- DRAM parameters (per-core shards, host-prepared) ----
    y1T = nc.dram_tensor("y1T", [BC, DM, Skv], F32, kind="ExternalInput")
    y2T = nc.dram_tensor("y2T", [BC, DM, Sq], F32, kind="ExternalInput")
    biasT = nc.dram_tensor("biasT", [HC, Skv, Sq], F32, kind="ExternalInput")
    wq = nc.dram_tensor("wq", [DM, DH], BF16, kind="ExternalInput")
    wk = nc.dram_tensor("wk", [DM, DH], BF16, kind="ExternalInput")
    wv = nc.dram_tensor("wv", [DM, DH], BF16, kind="ExternalInput")
    wo = nc.dram_tensor("wo", [DH, DM], BF16, kind="ExternalInput")
    bqv = nc.dram_tensor("bqv", [128, 2], F32, kind="ExternalInput")   # bq*SCALE, col-chunked
    bkv = nc.dram_tensor("bkv", [128, 2], F32, kind="ExternalInput")   # bk, col-chunked
    bvv = nc.dram_tensor("bvv", [1, DH], BF16, kind="ExternalInput")   # bv
    idn = nc.dram_tensor("idn", [128, 128], BF16, kind="ExternalInput")  # identity
    out = nc.dram_tensor("out", [BC, Sq, DM], BF16, kind="ExternalOutput")

    with tile.TileContext(nc) as tc, ExitStack() as ctx:
        build_kernel(ctx, tc, y1T, y2T, biasT, wq, wk, wv, wo, bqv, bkv, bvv, idn, out)

    nc.compile()
    return nc, "out"


def build_kernel(ctx, tc, y1T, y2T, biasT, wq, wk, wv, wo, bqv, bkv, bvv, idn, out):
    nc = tc.nc
    KT = DM // 128            # 8 contraction tiles for projections
    NQ = Sq // 512            # 2 moving-dim halves
    MS = Skv // 128           # 8 skv row tiles

    # ---------------- constant / weight loads ----------------
    consts = ctx.enter_context(tc.tile_pool(name="consts", bufs=1))

    wq_sb = consts.tile([128, KT, DH], BF16, tag="wq", name="wq")
    wk_sb = consts.tile([128, KT, DH], BF16, tag="wk", name="wk")
    wv_sb = consts.tile([128, KT, DH], BF16, tag="wv", name="wv")
    wo_sb = consts.tile([128, 2, DM], BF16, tag="wo", name="wo")   # [256,1024] -> 2 k-tiles
    for w_sb_, w_ in ((wq_sb, wq), (wk_sb, wk), (wv_sb, wv)):
        nc.sync.dma_start(
            out=w_sb_[:], in_=w_.ap().rearrange("(k p) m -> p k m", p=128)
        )
    nc.sync.dma_start(out=wo_sb[:], in_=wo.ap().rearrange("(k p) m -> p k m", p=128))

    def load_remaining_weights():
        pass

    bq_sb = consts.tile([128, 2], F32, tag="bq", name="bq")
    nc.sync.dma_start(out=bq_sb[:], in_=bqv[:, :])
    bk_sb = consts.tile([128, 2], F32, tag="bk", name="bk")
    nc.sync.dma_start(out=bk_sb[:], in_=bkv[:, :])
    bv_sb = consts.tile([1, DH], BF16, tag="bv", name="bv")
    nc.sync.dma_start(out=bv_sb[:], in_=bvv[:, :])
    ones_sb = consts.tile([1, 128], BF16, tag="ones", name="ones")
    nc.vector.memset(ones_sb[:], 1.0)
    idn_sb = consts.tile([128, 128], BF16, tag="idn", name="idn")
    nc.sync.dma_start(out=idn_sb[:], in_=idn[:, :])

    # persistent activations (both batches)
    acts = ctx.enter_context(tc.tile_pool(name="acts", bufs=1))
    qT_sb = [[acts.tile([128, Sq], BF16, tag=f"qT{b}{p}", name=f"qT{b}{p}") for p in range(2)]
             for b in range(BC)]                       # [b][head-pair] rows=2x64 dk
    kT_sb = [[acts.tile([128, Skv], BF16, tag=f"kT{b}{p}", name=f"kT{b}{p}") for p in range(2)]
             for b in range(BC)]
    v_sb = [[acts.tile([128, HC, 65], BF16, tag=f"v{b}{m}", name=f"v{b}{m}") for m in range(MS)]
            for b in range(BC)]                        # [b][skv-tile]: per-head [64 v | 1 ones]
    pair_sb = [[acts.tile([128, Sq], BF16, tag=f"at{b}{p}", name=f"at{b}{p}") for p in range(2)]
               for b in range(BC)]                     # attnT head-pair tiles

    # ---------------- Phase P: Q/K projections (V deferred into step 0) ----------------
    y1bf_all = {}
    ybf1 = ctx.enter_context(tc.tile_pool(name="ybf1", bufs=2 * KT))
    with ExitStack() as pctx:
        YR = int(os.environ.get("K_YRAW_BUFS", "4"))
        yraw = pctx.enter_context(tc.tile_pool(name="yraw", bufs=YR))
        YB = int(os.environ.get("K_YBF_BUFS", "8"))
        ybf = pctx.enter_context(tc.tile_pool(name="ybf", bufs=YB))
        ppsum = pctx.enter_context(tc.tile_pool(name="ppsum", bufs=4, space="PSUM"))

        for b in range(0 if SKIP_PROJ else BC):
            # --- load + cast y2T[b], then Q projection ---
            y2bf = []
            for t in range(KT):
                raw = yraw.tile([128, Sq], F32, tag="yraw", name="yraw")
                cast = ybf.tile([128, Sq], BF16, tag="ybf", name="ybf")
                for h in range(2):     # half-tile loads: finer arrival granularity
                    sl = slice(512 * h, 512 * (h + 1))
                    nc.sync.dma_start(out=raw[:, sl], in_=y2T[b, 128 * t:128 * (t + 1), sl])
                    (nc.gpsimd if (2 * t + h) % 2 else nc.vector).tensor_copy(
                        cast[:, sl], raw[:, sl])
                y2bf.append(cast)
            if b == 0:
                load_remaining_weights()
            qps = {(m, n): ppsum.tile([128, 512], F32, tag="ps", name="ps")
                   for m in range(2) for n in range(NQ)}
            for k in range(KT):           # k-outer: start as soon as tile k lands
                for m in range(2):
                    for n in range(NQ):
                        nc.tensor.matmul(
                            qps[(m, n)][:],
                            lhsT=wq_sb[:, k, 128 * m:128 * (m + 1)],
                            rhs=y2bf[k][:, 512 * n:512 * (n + 1)],
                            start=(k == 0), stop=(k == KT - 1),
                        )
            for m in range(2):
                for n in range(NQ):
                    nc.scalar.activation(
                        qT_sb[b][m][:, 512 * n:512 * (n + 1)], qps[(m, n)][:],
                        AF.Identity, bias=bq_sb[:, m:m + 1], scale=SCALE,
                    )
            # --- load + cast y1T[b], then K and V projections ---
            y1bf = []
            for t in range(KT):
                raw = yraw.tile([128, Skv], F32, tag="yraw", name="yraw")
                cast = ybf1.tile([128, Skv], BF16, tag="ybf1", name="ybf1")
                for h in range(2):
                    sl = slice(512 * h, 512 * (h + 1))
                    nc.sync.dma_start(out=raw[:, sl], in_=y1T[b, 128 * t:128 * (t + 1), sl])
                    (nc.gpsimd if (2 * t + h) % 2 else nc.vector).tensor_copy(
                        cast[:, sl], raw[:, sl])
                y1bf.append(cast)
            y1bf_all[b] = y1bf
            kps = {(m, n): ppsum.tile([128, 512], F32, tag="ps", name="ps")
                   for m in range(2) for n in range(NQ)}
            for k in range(KT):
                for m in range(2):
                    for n in range(NQ):
                        nc.tensor.matmul(
                            kps[(m, n)][:],
                            lhsT=wk_sb[:, k, 128 * m:128 * (m + 1)],
                            rhs=y1bf[k][:, 512 * n:512 * (n + 1)],
                            start=(k == 0), stop=(k == KT - 1),
                        )
            for m in range(2):
                for n in range(NQ):
                    nc.scalar.activation(
                        kT_sb[b][m][:, 512 * n:512 * (n + 1)], kps[(m, n)][:],
                        AF.Identity, bias=bk_sb[:, m:m + 1], scale=1.0,
                    )

    # ---------------- Phase A: attention + interleaved O projection ----------------
    with ExitStack() as actx:
        BP = int(os.environ.get("K_BIAS_BUFS", "16"))
        bpool = actx.enter_context(tc.tile_pool(name="bias", bufs=BP))
        lpsum = actx.enter_context(tc.tile_pool(name="lpsum", bufs=LPS_BUFS, space="PSUM"))
        apsum = actx.enter_context(tc.tile_pool(name="apsum", bufs=APS_BUFS, space="PSUM"))
        PPOOL_BUFS = int(os.environ.get("K_PPOOL_BUFS", "36"))
        ppool = actx.enter_context(tc.tile_pool(name="pT", bufs=PPOOL_BUFS))
        NP = int(os.environ.get("K_NORM_BUFS", "4"))
        npool = actx.enter_context(tc.tile_pool(name="norm", bufs=NP))
        rdram = actx.enter_context(tc.tile_pool(name="rdram", bufs=int(os.environ.get("K_RD_BUFS", "4")), space="DRAM"))
        opsum = actx.enter_context(tc.tile_pool(name="opsum", bufs=OPS_BUFS, space="PSUM"))
        opool = actx.enter_context(tc.tile_pool(name="osb", bufs=3))

        def emit_pv(b, h2, st, aps, kt):
            n_p, hp_p, pT_p = st
            h = 2 * hp_p + h2
            nc.tensor.matmul(
                aps[0:65, :],
                lhsT=v_sb[b][kt][:, h, :],
                rhs=pT_p[(b, kt, h2)][:],
                start=(kt == 0), stop=(kt == MS - 1),
            )

        def finalize_pv(b, h2, st, aps):
            n_p, hp_p, _ = st
            # r = 1/s (s = PSUM row 64, same base partition), then broadcast
            # across 64 partitions via DRAM round-trip (0-step partition AP)
            r_t = npool.tile([128, 512], F32, tag="r", name="r")
            nc.vector.reciprocal(r_t[64:65, :], aps[64:65, :])
            rd = rdram.tile([1, 512], F32, tag="rd", name="rd")
            nc.sync.dma_start(out=rd[:], in_=r_t[64:65, :])
            rd_ap = rd[:]
            rd_bcast = bass.AP(
                tensor=rd_ap.tensor,
                offset=rd_ap.offset,
                ap=[[0, 64], list(rd_ap.ap[-1])],
            )
            R_t = npool.tile([64, 512], F32, tag="R", name="R")
            nc.gpsimd.dma_start(out=R_t[:], in_=rd_bcast)
            dst = pair_sb[b][hp_p][64 * h2:64 * (h2 + 1),
                                   512 * n_p:512 * (n_p + 1)]
            if h2 == 0:
                nc.vector.scalar_tensor_tensor(
                    dst, aps[0:64, :], 1.0, R_t[:],
                    op0=ALU.mult, op1=ALU.mult,
                )
            else:
                tmp = npool.tile([64, 512], BF16, tag="atmp", name="atmp")
                nc.vector.scalar_tensor_tensor(
                    tmp[:], aps[0:64, :], 1.0, R_t[:],
                    op0=ALU.mult, op1=ALU.mult,
                )
                nc.sync.dma_start(out=dst, in_=tmp[:])

        def emit_o_half(n, only_b=None):
            # O projection for sq half n (overlaps the next attention step)
            for b in ((only_b,) if only_b is not None else range(BC)):
                for mt in range(4 * n, 4 * (n + 1)):
                    for no in range(DM // 512):
                        ps = opsum.tile([128, 512], F32, tag="o", name="o")
                        for kp in range(2):
                            nc.tensor.matmul(
                                ps[:],
                                lhsT=pair_sb[b][kp][:, 128 * mt:128 * (mt + 1)],
                                rhs=wo_sb[:, kp, 512 * no:512 * (no + 1)],
                                start=(kp == 0), stop=(kp == 1),
                            )
                        o_t = opool.tile([128, 512], BF16, tag="osb", name="osb")
                        nc.vector.tensor_copy(o_t[:], ps[:])
                        nc.sync.dma_start(
                            out=out[b, 128 * mt:128 * (mt + 1),
                                    512 * no:512 * (no + 1)],
                            in_=o_t[:],
                        )

        # software pipeline: PV of step i-1 interleaves with QK/exp of step i,
        # so PE never waits on the current step's ACT exps.
        SW_PIPE = bool(int(os.environ.get("K_SW_PIPE", "0")))
        steps = [(n, hp) for n in range(NQ) for hp in range(2)]
        prev = None            # (n, hp, pT) of the previous step
        pend_o = None          # sq half whose O projection is deferred
        for si, step in enumerate(steps + [None]):
            n, hp = step if step is not None else (None, None)
            if prev is not None:
                aps_t = {(b, h2): apsum.tile([128, 512], F32, tag="av", name="av")
                         for b in range(BC) for h2 in range(2)}
            if si < len(steps):
                bias_t = {}
                for kt in range(MS):
                    for h2 in range(2):
                        bt = bpool.tile([128, 512], F32, tag="bias", name="bias")
                        nc.sync.dma_start(
                            out=bt[:],
                            in_=biasT[2 * hp + h2,
                                      128 * kt:128 * (kt + 1),
                                      512 * n:512 * (n + 1)],
                        )
                        bbf = bpool.tile([128, 512], BF16, tag="biasbf", name="biasbf")
                        (nc.gpsimd if kt % 2 else nc.vector).tensor_copy(bbf[:], bt[:])
                        bias_t[(h2, kt)] = bbf
                pT = {}
                for b in range(BC):
                    for kt in range(MS):
                        for h2 in range(2):
                            lps = lpsum.tile([128, 512], F32, tag="l", name="l")
                            nc.tensor.matmul(
                                lps[:],
                                lhsT=kT_sb[b][hp][64 * h2:64 * (h2 + 1),
                                                  128 * kt:128 * (kt + 1)],
                                rhs=qT_sb[b][hp][64 * h2:64 * (h2 + 1),
                                                 512 * n:512 * (n + 1)],
                                start=True, stop=SKIP_BIAS,
                            )
                            if not SKIP_BIAS:
                                nc.tensor.matmul(
                                    lps[:],
                                    lhsT=idn_sb[:],
                                    rhs=bias_t[(h2, kt)][:],
                                    start=False, stop=True,
                                )
                            pt = ppool.tile([128, 512], BF16, tag="pT", name="pT")
                            nc.scalar.activation(pt[:], lps[:], AF.Exp)
                            pT[(b, kt, h2)] = pt
                        if prev is not None:
                            for h2 in range(2):   # previous step's PV, same kt
                                emit_pv(b, h2, prev, aps_t[(b, h2)], kt)
                if si == 0 and not SKIP_PROJ:
                    # deferred V projections: PE work that fills step 0's
                    # exp-drain window; v is only needed from the PV block on
                    for vb in range(BC):
                        for mt in range(MS):
                            vps = opsum.tile([128, DH], F32, tag="o", name="psv")
                            for k in range(KT):
                                nc.tensor.matmul(
                                    vps[:],
                                    lhsT=y1bf_all[vb][k][:, 128 * mt:128 * (mt + 1)],
                                    rhs=wv_sb[:, k, :],
                                    start=(k == 0), stop=False,
                                )
                            nc.tensor.matmul(   # + ones^T x bv (broadcast bias)
                                vps[:], lhsT=ones_sb[:, :], rhs=bv_sb[:, :],
                                start=False, stop=True,
                            )
                            nc.vector.tensor_copy(
                                v_sb[vb][mt][:, :, 0:64],
                                vps[:].rearrange("p (h d) -> p h d", d=DK),
                            )
                            nc.gpsimd.memset(v_sb[vb][mt][:, :, 64:65], 1.0)
                if not SW_PIPE:
                    prev = (n, hp, pT)
                    aps_t = {(b, h2): apsum.tile([128, 512], F32, tag="av", name="av")
                             for b in range(BC) for h2 in range(2)}
                    for b in range(BC):
                        for kt in range(MS):
                            for h2 in range(2):
                                emit_pv(b, h2, prev, aps_t[(b, h2)], kt)
            elif prev is not None:
                for b in range(BC):
                    for kt in range(MS):
                        for h2 in range(2):
                            emit_pv(b, h2, prev, aps_t[(b, h2)], kt)
            if prev is not None:
                n_p, hp_p, _ = prev
                for b in range(BC):
                    for h2 in range(2):
                        finalize_pv(b, h2, prev, aps_t[(b, h2)])
                if hp_p == 1:
                    emit_o_half(n_p)
            prev = (n, hp, pT) if (SW_PIPE and si < len(steps)) else None

# ====================== host wrapper ======================

def _prep_core_inputs(c, y1, y2, attn_bias, Wq, bq, Wk, bk, Wv, bv, Wo, bo):
    bp, hq = c // 4, c % 4
    bsl = slice(2 * bp, 2 * bp + 2)
    hsl = slice(DH * hq, DH * (hq + 1))
    bf16 = ml_dtypes.bfloat16
    f32 = np.float32
    return {
        "y1T": np.ascontiguousarray(y1[bsl].transpose(0, 2, 1), dtype=f32),
        "y2T": np.ascontiguousarray(y2[bsl].transpose(0, 2, 1), dtype=f32),
        "biasT": np.ascontiguousarray(
            attn_bias[0, 4 * hq:4 * hq + 4].transpose(0, 2, 1), dtype=f32
        ),
        "wq": np.ascontiguousarray(Wq[:, hsl]).astype(bf16),
        "wk": np.ascontiguousarray(Wk[:, hsl]).astype(bf16),
        "wv": np.ascontiguousarray(Wv[:, hsl]).astype(bf16),
        "wo": np.ascontiguousarray(Wo[hsl, :]).astype(bf16),
        "bqv": np.ascontiguousarray(
            (bq[hsl].astype(f32) * SCALE).reshape(2, 128).T
        ),
        "bkv": np.ascontiguousarray(bk[hsl].astype(f32).reshape(2, 128).T),
        "bvv": bv[hsl].astype(bf16).reshape(1, DH),
        "idn": np.eye(128, dtype=f32).astype(bf16),
    }


def kernel(y1, y2, attn_bias, Wq, bq, Wk, bk, Wv, bv, Wo, bo):
    global _PROGRAM, LAST_RESULTS
    args = [np.asarray(x) for x in
            (y1, y2, attn_bias, Wq, bq, Wk, bk, Wv, bv, Wo, bo)]
    if _PROGRAM is None:
        _PROGRAM = build_program()
    nc, out_name = _PROGRAM

    in_maps = [_prep_core_inputs(c, *args) for c in range(N_CORES)]
    res = run_bass_kernel_spmd(nc, in_maps, list(range(N_CORES)), trace=TRACE)
    LAST_RESULTS = res

    out = np.zeros((B, Sq, DM), np.float32)
    for c in range(N_CORES):
        part = np.asarray(res.results[c][out_name]).astype(np.float32)
        bp = c // 4
        out[2 * bp] += part[0]
        out[2 * bp + 1] += part[1]
    out += np.asarray(args[10]).astype(np.float32)[None, None, :]
    return out

